# revision 20
# baseline (speedup 1.0000x reference)
# Involution2d (K=7) Trainium2 kernel — 8-core SPMD, batch+spatial sharding.
#
# V4: TensorE diag-matmul involution (V3.2) with a halved input payload.
# Per 2-row pixel block t the involution is 49 x 2 accumulating [64x64]
# diag-matmuls in fp32 PSUM:
#   out_T[64*rho + j, c] += ker[o, row 2t+rho, j] * xT[src_pixel, c]
# The two [64x64] halves of consecutive matmuls land on alternating PE
# row-groups, so each half's LDWEIGHTS hides under the other's matmul
# (measured 35 ns/MM, zero gaps).
# New in V4: only SEVEN shifted transposed-x copies (one per column shift
# dj) instead of 14 — the row-shift parity is absorbed by tile_position:
# an odd row shift sh makes output half rho read source half
# rho'' = (rho+sh)%2, i.e. lhsT/rhs sit at partition half rho'' while the
# output sits at half rho (row-group != col-group, which the PE supports).
# The diag values for that case must live at the opposite partition half
# from kerT, so one partition-swapped SBUF->SBUF DMA (kerTswap) feeds the
# odd-shift diag builds.  Offsets are host-permuted (even-shift first) so
# both variants build with one batched broadcast-AP tensor_mul each, plus
# a few ScalarE singles.  Front latency drops from ~40us (8.7 MB DMA) to
# ~16us (4.6 MB DMA overlapped with generation).
import numpy as np

EPS = 1e-5
KK = 7
C = 128
H = 64
W = 64
B = 4
HH = 32              # rows per core
P = HH * W           # 2048 output pixels per core
NBLK = 16            # 128-pixel blocks per core
XBLK = 20            # blocks per shifted-x copy (offsets -2..+2 around 16)
GEN_CHUNK = 512

# offset permutation: even row-shift (di in {1,3,5}) first, odd after.
# pi (permuted index) is the kerT/maskT/d_all column; o = PERM[pi] gives
# geometry.  NEVEN = 21 even-shift offsets.
PERM = sorted(range(49), key=lambda o: ((o // 7 - 3) & 1, o))
NEVEN = sum(1 for o in range(49) if ((o // 7 - 3) & 1) == 0)  # 21
NODD = 49 - NEVEN                                             # 28
NSE = 9              # of the odd ones, how many build on ScalarE
NDVODD = NODD - NSE  # odd diags built in the batched VectorE op

_STATE = {}


def _build():
    import concourse.tile as tile
    from concourse import bacc, mybir

    f32 = mybir.dt.float32
    f16 = mybir.dt.float16
    nc = bacc.Bacc("TRN2", target_bir_lowering=False, debug=False)

    xt_d = nc.dram_tensor("xt7", [C, 7 * XBLK * 128], f16, kind="ExternalInput").ap()
    xn_d = nc.dram_tensor("xn", [C, P], f16, kind="ExternalInput").ap()
    w1sT_d = nc.dram_tensor("w1sT", [C, 32], f16, kind="ExternalInput").ap()
    b1f_d = nc.dram_tensor("b1f", [32, 1], f32, kind="ExternalInput").ap()
    w2Ta_d = nc.dram_tensor("w2Ta", [33, 49], f16, kind="ExternalInput").ap()
    maskT_d = nc.dram_tensor("maskT", [C, NBLK * 49], f16, kind="ExternalInput").ap()
    eye_d = nc.dram_tensor("eyec", [C, 64], f16, kind="ExternalInput").ap()
    out_d = nc.dram_tensor("out", [C, P], f16, kind="ExternalOutput").ap()
    out2_d = nc.dram_tensor("out2", [C, P], f16, kind="ExternalOutput").ap()

    with tile.TileContext(nc) as tc:
        with (
            tc.tile_pool(name="consts", bufs=1) as cpool,
            tc.tile_pool(name="dpool", bufs=6) as dpool,
            tc.tile_pool(name="pgen", bufs=2, space="PSUM") as pgen,
            tc.tile_pool(name="pout", bufs=2, space="PSUM") as pout,
        ):
            xn = cpool.tile([C, P], f16, tag="xn")
            nc.sync.dma_start(xn[:], xn_d)
            w1sT = cpool.tile([C, 32], f16, tag="w1")
            nc.sync.dma_start(w1sT[:], w1sT_d)
            b1f = cpool.tile([32, 1], f32, tag="b1")
            nc.sync.dma_start(b1f[:], b1f_d)
            w2Ta = cpool.tile([33, 49], f16, tag="w2")
            nc.sync.dma_start(w2Ta[:], w2Ta_d)
            maskT = cpool.tile([C, NBLK * 49], f16, tag="mask")
            nc.sync.dma_start(maskT[:], maskT_d)
            eye = cpool.tile([C, 64], f16, tag="eye")
            nc.sync.dma_start(eye[:], eye_d)
            xtc = []
            for cpy in range(7):
                xt_t = cpool.tile([C, XBLK * 128], f16, tag=f"xtc{cpy}")
                nc.sync.dma_start(
                    xt_t[:], xt_d[:, cpy * XBLK * 128:(cpy + 1) * XBLK * 128]
                )
                xtc.append(xt_t)

            f_aug = cpool.tile([33, P], f16, tag="f")
            nc.vector.memset(f_aug[32:33, :], 1.0)
            kerT = cpool.tile([C, NBLK * 49], f32, tag="kerT")
            kerTs = cpool.tile([C, NBLK * 49], f32, tag="kerTs")
            outT = cpool.tile([C, P], f16, tag="outT")
            outT2 = cpool.tile([C, P], f16, tag="outT2")

            # ---- kernel generation: f = relu(w1s^T x + b1) ----
            for ci in range(P // GEN_CHUNK):
                sl = slice(ci * GEN_CHUNK, (ci + 1) * GEN_CHUNK)
                f1 = pgen.tile([32, GEN_CHUNK], f32, tag="f1")
                nc.tensor.matmul(f1[:], w1sT[:], xn[:, sl], start=True, stop=True)
                nc.scalar.activation(
                    f_aug[0:32, sl], f1[:], mybir.ActivationFunctionType.Relu,
                    bias=b1f[:],
                )

            # ---- kerT[q, pi] per block: (f_blk^T w2 + b2) * maskT ----
            for blk in range(NBLK):
                ksl = slice(blk * 49, (blk + 1) * 49)
                kps = pgen.tile([C, 512], f32, tag="kps")
                nc.tensor.matmul(
                    kps[:, 0:49], f_aug[:, blk * 128:(blk + 1) * 128], w2Ta[:],
                    start=True, stop=True,
                )
                nc.vector.tensor_mul(kerT[:, ksl], kps[:, 0:49], maskT[:, ksl])

            # ---- kerTswap: kerT with partition halves swapped (whole
            # tile; only the odd-shift columns are consumed) ----
            for h in range(2):
                nc.sync.dma_start(
                    kerTs[64 * h:64 * h + 64, :],
                    kerT[64 * (1 - h):64 * (1 - h) + 64, :],
                )

            # ---- involution: per block, batched diag builds + 49x2
            # accumulating [64x64] diag-matmuls ----
            for blk in range(NBLK):
                ops = pout.tile([C, 512], f32, tag="ops")
                opsw = pout.tile([C, 512], f32, tag="opsw")
                d_all = dpool.tile([C, 49 * 64], f16, tag="dall")
                nc.vector.tensor_mul(
                    d_all[:, 0:NEVEN * 64].rearrange("k (o j) -> k o j", j=64),
                    eye[:].unsqueeze(1).broadcast_to([C, NEVEN, 64]),
                    kerT[:, blk * 49: blk * 49 + NEVEN].unsqueeze(2)
                        .broadcast_to([C, NEVEN, 64]),
                )
                nc.gpsimd.tensor_mul(
                    d_all[:, NEVEN * 64:(NEVEN + NDVODD) * 64]
                        .rearrange("k (o j) -> k o j", j=64),
                    eye[:].unsqueeze(1).broadcast_to([C, NDVODD, 64]),
                    kerTs[:, blk * 49 + NEVEN: blk * 49 + NEVEN + NDVODD]
                        .unsqueeze(2).broadcast_to([C, NDVODD, 64]),
                )
                nc.vector.tensor_mul(
                    d_all[:, (NEVEN + NDVODD) * 64: 49 * 64]
                        .rearrange("k (o j) -> k o j", j=64),
                    eye[:].unsqueeze(1).broadcast_to([C, NSE, 64]),
                    kerTs[:, blk * 49 + NEVEN + NDVODD: blk * 49 + 49]
                        .unsqueeze(2).broadcast_to([C, NSE, 64]),
                )
                for pi in range(49):
                    o = PERM[pi]
                    di, dj = divmod(o, 7)
                    sh = di - 3
                    odd = pi >= NEVEN
                    tgt = opsw if odd else ops
                    for rho in range(2):
                        rpp = (rho + sh) & 1
                        u = (rho + sh - rpp) >> 1     # in {-2..2}
                        xoff = (blk + u + 2) * 128
                        nc.tensor.matmul(
                            tgt[64 * rpp:64 * rpp + 64, 0:128],
                            d_all[64 * rpp:64 * rpp + 64,
                                  pi * 64:(pi + 1) * 64],
                            xtc[dj][64 * rpp:64 * rpp + 64,
                                    xoff: xoff + 128],
                            start=(pi == (NEVEN if odd else 0)),
                            stop=(pi == (48 if odd else NEVEN - 1)),
                            tile_position=(64 * rpp, 64 * rpp),
                        )
                nc.vector.tensor_copy(
                    outT[:, blk * 128:(blk + 1) * 128], ops[:, 0:128]
                )
                nc.vector.tensor_copy(
                    outT2[:, blk * 128:(blk + 1) * 128], opsw[:, 0:128]
                )

            nc.sync.dma_start(out_d, outT[:])
            nc.sync.dma_start(out2_d, outT2[:])

    nc.compile()
    return nc


def _get_nc():
    if "nc" not in _STATE:
        _STATE["nc"] = _build()
    return _STATE["nc"]


def _host_prep(x, w1, b1, bn_gamma, bn_beta, bn_mean, bn_var, w2, b2):
    x = np.asarray(x, dtype=np.float32)
    scale = np.asarray(bn_gamma) / np.sqrt(np.asarray(bn_var) + EPS)
    w1s = (np.asarray(w1) * scale[:, None]).astype(np.float32)
    b1f = (np.asarray(b1) * scale + np.asarray(bn_beta)
           - np.asarray(bn_mean) * scale).astype(np.float32)
    w1sT = np.ascontiguousarray(w1s.T.astype(np.float16))        # [128, 32]
    b1fc = np.ascontiguousarray(b1f[:, None].astype(np.float32))
    w2Ta_full = np.zeros((33, 49), dtype=np.float16)
    w2Ta_full[:32] = np.asarray(w2, np.float32).T.astype(np.float16)
    w2Ta_full[32] = np.asarray(b2, np.float32).astype(np.float16)
    w2Ta = np.ascontiguousarray(w2Ta_full[:, PERM])              # permuted

    # maskT[k, blk*49 + pi]: kernel zeroed where w + dj leaves the row
    wcol = np.arange(P, dtype=np.int64) % W
    mask = np.zeros((49, P), dtype=np.float16)
    for ipp in range(KK):
        for jpp in range(KK):
            dj = jpp - 3
            mask[ipp * KK + jpp] = ((wcol + dj >= 0) & (wcol + dj < W))
    mask = mask[PERM]                                            # permuted
    maskT = np.ascontiguousarray(
        mask.reshape(49, NBLK, 128).transpose(2, 1, 0).reshape(C, NBLK * 49)
    )

    eye = np.zeros((C, 64), dtype=np.float16)
    eye[np.arange(C), np.arange(C) % 64] = 1.0

    in_maps = []
    for core in range(8):
        b, half = divmod(core, 2)
        h0 = HH * half
        xn = np.ascontiguousarray(
            x[b, :, h0:h0 + HH, :].reshape(C, P).astype(np.float16)
        )
        # 7 shifted transposed copies: xt7[k, cpy*XBLK*128 + ib*128 + c]
        # holds x[b, c, strip_pixel (ib-2)*128 + k + (dj-3)]
        PAD = 6 * W
        xpadT = np.zeros((PAD + H * W + PAD, C), dtype=np.float16)
        xpadT[PAD:PAD + H * W] = x[b].reshape(C, H * W).T.astype(np.float16)
        xt7 = np.zeros((C, 7 * XBLK * 128), dtype=np.float16)
        base0 = PAD + h0 * W
        for dj in range(KK):
            st = base0 - 2 * 128 + (dj - 3)
            seg = xpadT[st: st + XBLK * 128]          # [XBLK*128, C]
            seg = np.ascontiguousarray(
                seg.reshape(XBLK, 128, C).transpose(1, 0, 2)
            )
            xt7[:, dj * XBLK * 128:(dj + 1) * XBLK * 128] = (
                seg.reshape(128, XBLK * 128)
            )
        in_maps.append({
            "xt7": xt7, "xn": xn, "w1sT": w1sT, "b1f": b1fc,
            "w2Ta": w2Ta, "maskT": maskT, "eyec": eye,
        })
    return in_maps


def run(inputs: dict, trace: bool = False):
    from concourse.bass_utils import run_bass_kernel_spmd

    nc = _get_nc()
    in_maps = _host_prep(**inputs)
    res = run_bass_kernel_spmd(
        nc, in_maps, core_ids=list(range(8)), trace=trace,
    )
    out = np.zeros((B, C, H, W), dtype=np.float32)
    for core in range(8):
        b, half = divmod(core, 2)
        h0 = HH * half
        arr = (res.results[core]["out"].astype(np.float32)
               + np.roll(res.results[core]["out2"].astype(np.float32),
                         64, axis=0))                  # [q, blk*128+c]
        arr = arr.reshape(128, NBLK, 128).transpose(1, 0, 2).reshape(P, C)
        out[b, :, h0:h0 + HH, :] = arr.T.reshape(C, HH, W)
    return out, res


def kernel(**inputs) -> np.ndarray:
    out, _ = run(inputs, trace=False)
    return out


# revision 21
# speedup vs baseline: 1.2482x; 1.2482x over previous
# Involution2d (K=7) Trainium2 kernel — 8-core SPMD, batch+spatial sharding.
#
# V4: TensorE diag-matmul involution (V3.2) with a halved input payload.
# Per 2-row pixel block t the involution is 49 x 2 accumulating [64x64]
# diag-matmuls in fp32 PSUM:
#   out_T[64*rho + j, c] += ker[o, row 2t+rho, j] * xT[src_pixel, c]
# The two [64x64] halves of consecutive matmuls land on alternating PE
# row-groups, so each half's LDWEIGHTS hides under the other's matmul
# (measured 35 ns/MM, zero gaps).
# New in V4: only SEVEN shifted transposed-x copies (one per column shift
# dj) instead of 14 — the row-shift parity is absorbed by tile_position:
# an odd row shift sh makes output half rho read source half
# rho'' = (rho+sh)%2, i.e. lhsT/rhs sit at partition half rho'' while the
# output sits at half rho (row-group != col-group, which the PE supports).
# The diag values for that case must live at the opposite partition half
# from kerT, so one partition-swapped SBUF->SBUF DMA (kerTswap) feeds the
# odd-shift diag builds.  Offsets are host-permuted (even-shift first) so
# both variants build with one batched broadcast-AP tensor_mul each, plus
# a few ScalarE singles.  Front latency drops from ~40us (8.7 MB DMA) to
# ~16us (4.6 MB DMA overlapped with generation).
import numpy as np

EPS = 1e-5
KK = 7
C = 128
H = 64
W = 64
B = 4
HH = 32              # rows per core
P = HH * W           # 2048 output pixels per core
NBLK = 16            # 128-pixel blocks per core
XBLK = 20            # blocks per shifted-x copy (offsets -2..+2 around 16)
GEN_CHUNK = 512

# offset permutation: even row-shift (di in {1,3,5}) first, odd after.
# pi (permuted index) is the kerT/maskT/d_all column; o = PERM[pi] gives
# geometry.  NEVEN = 21 even-shift offsets.
PERM = sorted(range(49), key=lambda o: ((o // 7 - 3) & 1, o))
NEVEN = sum(1 for o in range(49) if ((o // 7 - 3) & 1) == 0)  # 21
NODD = 49 - NEVEN                                             # 28
NSE = 9              # of the odd ones, how many build on ScalarE
NDVODD = NODD - NSE  # odd diags built in the batched VectorE op

_STATE = {}


def _build():
    import concourse.tile as tile
    from concourse import bacc, mybir

    f32 = mybir.dt.float32
    f16 = mybir.dt.float16
    nc = bacc.Bacc("TRN2", target_bir_lowering=False, debug=False)

    xt_d = nc.dram_tensor("xt7", [C, 7 * XBLK * 128], f16, kind="ExternalInput").ap()
    xn_d = nc.dram_tensor("xn", [C, P], f16, kind="ExternalInput").ap()
    w1sT_d = nc.dram_tensor("w1sT", [C, 32], f16, kind="ExternalInput").ap()
    b1f_d = nc.dram_tensor("b1f", [32, 1], f32, kind="ExternalInput").ap()
    w2Ta_d = nc.dram_tensor("w2Ta", [33, 49], f16, kind="ExternalInput").ap()
    maskT_d = nc.dram_tensor("maskT", [C, NBLK * 49], f16, kind="ExternalInput").ap()
    eye_d = nc.dram_tensor("eyec", [C, 64], f16, kind="ExternalInput").ap()
    out_d = nc.dram_tensor("out", [C, P], f16, kind="ExternalOutput").ap()
    out2_d = nc.dram_tensor("out2", [C, P], f16, kind="ExternalOutput").ap()

    with tile.TileContext(nc) as tc:
        with (
            tc.tile_pool(name="consts", bufs=1) as cpool,
            tc.tile_pool(name="dpool", bufs=6) as dpool,
            tc.tile_pool(name="pgen", bufs=2, space="PSUM") as pgen,
            tc.tile_pool(name="pout", bufs=2, space="PSUM") as pout,
        ):
            xn = cpool.tile([C, P], f16, tag="xn")
            nc.sync.dma_start(xn[:], xn_d)
            w1sT = cpool.tile([C, 32], f16, tag="w1")
            nc.sync.dma_start(w1sT[:], w1sT_d)
            b1f = cpool.tile([32, 1], f32, tag="b1")
            nc.sync.dma_start(b1f[:], b1f_d)
            w2Ta = cpool.tile([33, 49], f16, tag="w2")
            nc.sync.dma_start(w2Ta[:], w2Ta_d)
            maskT = cpool.tile([C, NBLK * 49], f16, tag="mask")
            nc.sync.dma_start(maskT[:], maskT_d)
            eye = cpool.tile([C, 64], f16, tag="eye")
            nc.sync.dma_start(eye[:], eye_d)
            xtc = []
            for cpy in range(7):
                xt_t = cpool.tile([C, XBLK * 128], f16, tag=f"xtc{cpy}")
                nc.sync.dma_start(
                    xt_t[:], xt_d[:, cpy * XBLK * 128:(cpy + 1) * XBLK * 128]
                )
                xtc.append(xt_t)

            f_aug = cpool.tile([33, P], f16, tag="f")
            nc.vector.memset(f_aug[32:33, :], 1.0)
            kerT = cpool.tile([C, NBLK * 49], f32, tag="kerT")
            kerTs = cpool.tile([C, NBLK * 49], f32, tag="kerTs")
            outT = cpool.tile([C, P], f16, tag="outT")
            outT2 = cpool.tile([C, P], f16, tag="outT2")

            # ---- kernel generation: f = relu(w1s^T x + b1) ----
            for ci in range(P // GEN_CHUNK):
                sl = slice(ci * GEN_CHUNK, (ci + 1) * GEN_CHUNK)
                f1 = pgen.tile([32, GEN_CHUNK], f32, tag="f1")
                nc.tensor.matmul(f1[:], w1sT[:], xn[:, sl], start=True, stop=True)
                nc.scalar.activation(
                    f_aug[0:32, sl], f1[:], mybir.ActivationFunctionType.Relu,
                    bias=b1f[:],
                )

            # ---- kerT[q, pi] per block: (f_blk^T w2 + b2) * maskT ----
            for blk in range(NBLK):
                ksl = slice(blk * 49, (blk + 1) * 49)
                kps = pgen.tile([C, 512], f32, tag="kps")
                nc.tensor.matmul(
                    kps[:, 0:49], f_aug[:, blk * 128:(blk + 1) * 128], w2Ta[:],
                    start=True, stop=True,
                )
                nc.vector.tensor_mul(kerT[:, ksl], kps[:, 0:49], maskT[:, ksl])

            # ---- kerTswap: kerT with partition halves swapped (whole
            # tile; only the odd-shift columns are consumed) ----
            for h in range(2):
                nc.sync.dma_start(
                    kerTs[64 * h:64 * h + 64, :],
                    kerT[64 * (1 - h):64 * (1 - h) + 64, :],
                )

            # ---- involution: per block, batched diag builds + 49x2
            # accumulating [64x64] diag-matmuls ----
            for blk in range(NBLK):
                ops = pout.tile([C, 512], f32, tag="ops")
                opsw = pout.tile([C, 512], f32, tag="opsw")
                d_all = dpool.tile([C, 49 * 64], f16, tag="dall")
                nc.vector.tensor_mul(
                    d_all[:, 0:NEVEN * 64].rearrange("k (o j) -> k o j", j=64),
                    eye[:].unsqueeze(1).broadcast_to([C, NEVEN, 64]),
                    kerT[:, blk * 49: blk * 49 + NEVEN].unsqueeze(2)
                        .broadcast_to([C, NEVEN, 64]),
                )
                nc.vector.tensor_mul(
                    d_all[:, NEVEN * 64:(NEVEN + NDVODD) * 64]
                        .rearrange("k (o j) -> k o j", j=64),
                    eye[:].unsqueeze(1).broadcast_to([C, NDVODD, 64]),
                    kerTs[:, blk * 49 + NEVEN: blk * 49 + NEVEN + NDVODD]
                        .unsqueeze(2).broadcast_to([C, NDVODD, 64]),
                )
                for pi in range(NEVEN + NDVODD, 49):
                    nc.scalar.activation(
                        d_all[:, pi * 64:(pi + 1) * 64], eye[:],
                        mybir.ActivationFunctionType.Copy,
                        scale=kerTs[:, blk * 49 + pi: blk * 49 + pi + 1],
                    )
                for pi in range(49):
                    o = PERM[pi]
                    di, dj = divmod(o, 7)
                    sh = di - 3
                    odd = pi >= NEVEN
                    tgt = opsw if odd else ops
                    for rho in range(2):
                        rpp = (rho + sh) & 1
                        u = (rho + sh - rpp) >> 1     # in {-2..2}
                        xoff = (blk + u + 2) * 128
                        nc.tensor.matmul(
                            tgt[64 * rpp:64 * rpp + 64, 0:128],
                            d_all[64 * rpp:64 * rpp + 64,
                                  pi * 64:(pi + 1) * 64],
                            xtc[dj][64 * rpp:64 * rpp + 64,
                                    xoff: xoff + 128],
                            start=(pi == (NEVEN if odd else 0)),
                            stop=(pi == (48 if odd else NEVEN - 1)),
                            tile_position=(64 * rpp, 64 * rpp),
                        )
                nc.vector.tensor_copy(
                    outT[:, blk * 128:(blk + 1) * 128], ops[:, 0:128]
                )
                nc.vector.tensor_copy(
                    outT2[:, blk * 128:(blk + 1) * 128], opsw[:, 0:128]
                )

            nc.sync.dma_start(out_d, outT[:])
            nc.sync.dma_start(out2_d, outT2[:])

    nc.compile()
    return nc


def _get_nc():
    if "nc" not in _STATE:
        _STATE["nc"] = _build()
    return _STATE["nc"]


def _host_prep(x, w1, b1, bn_gamma, bn_beta, bn_mean, bn_var, w2, b2):
    x = np.asarray(x, dtype=np.float32)
    scale = np.asarray(bn_gamma) / np.sqrt(np.asarray(bn_var) + EPS)
    w1s = (np.asarray(w1) * scale[:, None]).astype(np.float32)
    b1f = (np.asarray(b1) * scale + np.asarray(bn_beta)
           - np.asarray(bn_mean) * scale).astype(np.float32)
    w1sT = np.ascontiguousarray(w1s.T.astype(np.float16))        # [128, 32]
    b1fc = np.ascontiguousarray(b1f[:, None].astype(np.float32))
    w2Ta_full = np.zeros((33, 49), dtype=np.float16)
    w2Ta_full[:32] = np.asarray(w2, np.float32).T.astype(np.float16)
    w2Ta_full[32] = np.asarray(b2, np.float32).astype(np.float16)
    w2Ta = np.ascontiguousarray(w2Ta_full[:, PERM])              # permuted

    # maskT[k, blk*49 + pi]: kernel zeroed where w + dj leaves the row
    wcol = np.arange(P, dtype=np.int64) % W
    mask = np.zeros((49, P), dtype=np.float16)
    for ipp in range(KK):
        for jpp in range(KK):
            dj = jpp - 3
            mask[ipp * KK + jpp] = ((wcol + dj >= 0) & (wcol + dj < W))
    mask = mask[PERM]                                            # permuted
    maskT = np.ascontiguousarray(
        mask.reshape(49, NBLK, 128).transpose(2, 1, 0).reshape(C, NBLK * 49)
    )

    eye = np.zeros((C, 64), dtype=np.float16)
    eye[np.arange(C), np.arange(C) % 64] = 1.0

    in_maps = []
    for core in range(8):
        b, half = divmod(core, 2)
        h0 = HH * half
        xn = np.ascontiguousarray(
            x[b, :, h0:h0 + HH, :].reshape(C, P).astype(np.float16)
        )
        # 7 shifted transposed copies: xt7[k, cpy*XBLK*128 + ib*128 + c]
        # holds x[b, c, strip_pixel (ib-2)*128 + k + (dj-3)]
        PAD = 6 * W
        xpadT = np.zeros((PAD + H * W + PAD, C), dtype=np.float16)
        xpadT[PAD:PAD + H * W] = x[b].reshape(C, H * W).T.astype(np.float16)
        xt7 = np.zeros((C, 7 * XBLK * 128), dtype=np.float16)
        base0 = PAD + h0 * W
        for dj in range(KK):
            st = base0 - 2 * 128 + (dj - 3)
            seg = xpadT[st: st + XBLK * 128]          # [XBLK*128, C]
            seg = np.ascontiguousarray(
                seg.reshape(XBLK, 128, C).transpose(1, 0, 2)
            )
            xt7[:, dj * XBLK * 128:(dj + 1) * XBLK * 128] = (
                seg.reshape(128, XBLK * 128)
            )
        in_maps.append({
            "xt7": xt7, "xn": xn, "w1sT": w1sT, "b1f": b1fc,
            "w2Ta": w2Ta, "maskT": maskT, "eyec": eye,
        })
    return in_maps


def run(inputs: dict, trace: bool = False):
    from concourse.bass_utils import run_bass_kernel_spmd

    nc = _get_nc()
    in_maps = _host_prep(**inputs)
    res = run_bass_kernel_spmd(
        nc, in_maps, core_ids=list(range(8)), trace=trace,
    )
    out = np.zeros((B, C, H, W), dtype=np.float32)
    for core in range(8):
        b, half = divmod(core, 2)
        h0 = HH * half
        arr = (res.results[core]["out"].astype(np.float32)
               + np.roll(res.results[core]["out2"].astype(np.float32),
                         64, axis=0))                  # [q, blk*128+c]
        arr = arr.reshape(128, NBLK, 128).transpose(1, 0, 2).reshape(P, C)
        out[b, :, h0:h0 + HH, :] = arr.T.reshape(C, HH, W)
    return out, res


def kernel(**inputs) -> np.ndarray:
    out, _ = run(inputs, trace=False)
    return out


# revision 23
# speedup vs baseline: 1.2576x; 1.0076x over previous
# Involution2d (K=7) Trainium2 kernel — 8-core SPMD, batch+spatial sharding.
#
# V4: TensorE diag-matmul involution (V3.2) with a halved input payload.
# Per 2-row pixel block t the involution is 49 x 2 accumulating [64x64]
# diag-matmuls in fp32 PSUM:
#   out_T[64*rho + j, c] += ker[o, row 2t+rho, j] * xT[src_pixel, c]
# The two [64x64] halves of consecutive matmuls land on alternating PE
# row-groups, so each half's LDWEIGHTS hides under the other's matmul
# (measured 35 ns/MM, zero gaps).
# New in V4: only SEVEN shifted transposed-x copies (one per column shift
# dj) instead of 14 — the row-shift parity is absorbed by tile_position:
# an odd row shift sh makes output half rho read source half
# rho'' = (rho+sh)%2, i.e. lhsT/rhs sit at partition half rho'' while the
# output sits at half rho (row-group != col-group, which the PE supports).
# The diag values for that case must live at the opposite partition half
# from kerT, so one partition-swapped SBUF->SBUF DMA (kerTswap) feeds the
# odd-shift diag builds.  Offsets are host-permuted (even-shift first) so
# both variants build with one batched broadcast-AP tensor_mul each, plus
# a few ScalarE singles.  Front latency drops from ~40us (8.7 MB DMA) to
# ~16us (4.6 MB DMA overlapped with generation).
import numpy as np

EPS = 1e-5
KK = 7
C = 128
H = 64
W = 64
B = 4
HH = 32              # rows per core
P = HH * W           # 2048 output pixels per core
NBLK = 16            # 128-pixel blocks per core
XBLK = 20            # blocks per shifted-x copy (offsets -2..+2 around 16)
GEN_CHUNK = 512

# offset permutation: even row-shift (di in {1,3,5}) first, odd after.
# pi (permuted index) is the kerT/maskT/d_all column; o = PERM[pi] gives
# geometry.  NEVEN = 21 even-shift offsets.
PERM = sorted(range(49), key=lambda o: ((o // 7 - 3) & 1, o))
NEVEN = sum(1 for o in range(49) if ((o // 7 - 3) & 1) == 0)  # 21
NODD = 49 - NEVEN                                             # 28
NSE = 9              # of the odd ones, how many build on ScalarE
NDVODD = NODD - NSE  # odd diags built in the batched VectorE op

_STATE = {}


def _build():
    import concourse.tile as tile
    from concourse import bacc, mybir

    f32 = mybir.dt.float32
    f16 = mybir.dt.float16
    nc = bacc.Bacc("TRN2", target_bir_lowering=False, debug=False)

    xt_d = nc.dram_tensor("xt7", [C, 7 * XBLK * 128], f16, kind="ExternalInput").ap()
    xn_d = nc.dram_tensor("xn", [C, P], f16, kind="ExternalInput").ap()
    w1sT_d = nc.dram_tensor("w1sT", [C, 32], f16, kind="ExternalInput").ap()
    b1f_d = nc.dram_tensor("b1f", [32, 1], f32, kind="ExternalInput").ap()
    w2Ta_d = nc.dram_tensor("w2Ta", [33, 49], f16, kind="ExternalInput").ap()
    maskT_d = nc.dram_tensor("maskT", [C, NBLK * 49], f16, kind="ExternalInput").ap()
    eye_d = nc.dram_tensor("eyec", [C, 64], f16, kind="ExternalInput").ap()
    out_d = nc.dram_tensor("out", [C, P], f16, kind="ExternalOutput").ap()
    out2_d = nc.dram_tensor("out2", [C, P], f16, kind="ExternalOutput").ap()

    with tile.TileContext(nc) as tc:
        with (
            tc.tile_pool(name="consts", bufs=1) as cpool,
            tc.tile_pool(name="dpool", bufs=6) as dpool,
            tc.tile_pool(name="pgen", bufs=2, space="PSUM") as pgen,
            tc.tile_pool(name="pout", bufs=2, space="PSUM") as pout,
        ):
            xn = cpool.tile([C, P], f16, tag="xn")
            nc.sync.dma_start(xn[:], xn_d)
            w1sT = cpool.tile([C, 32], f16, tag="w1")
            nc.sync.dma_start(w1sT[:], w1sT_d)
            b1f = cpool.tile([32, 1], f32, tag="b1")
            nc.sync.dma_start(b1f[:], b1f_d)
            w2Ta = cpool.tile([33, 49], f16, tag="w2")
            nc.sync.dma_start(w2Ta[:], w2Ta_d)
            maskT = cpool.tile([C, NBLK * 49], f16, tag="mask")
            nc.sync.dma_start(maskT[:], maskT_d)
            eye = cpool.tile([C, 64], f16, tag="eye")
            nc.sync.dma_start(eye[:], eye_d)
            xtc = []
            for cpy in range(7):
                xt_t = cpool.tile([C, XBLK * 128], f16, tag=f"xtc{cpy}")
                nc.sync.dma_start(
                    xt_t[:], xt_d[:, cpy * XBLK * 128:(cpy + 1) * XBLK * 128]
                )
                xtc.append(xt_t)

            f_aug = cpool.tile([33, P], f16, tag="f")
            nc.vector.memset(f_aug[32:33, :], 1.0)
            kerT = cpool.tile([C, NBLK * 49], f32, tag="kerT")
            kerTs = cpool.tile([C, NBLK * 49], f32, tag="kerTs")
            outT = cpool.tile([C, P], f16, tag="outT")
            outT2 = cpool.tile([C, P], f16, tag="outT2")

            # ---- kernel generation: f = relu(w1s^T x + b1) ----
            for ci in range(P // GEN_CHUNK):
                sl = slice(ci * GEN_CHUNK, (ci + 1) * GEN_CHUNK)
                f1 = pgen.tile([32, GEN_CHUNK], f32, tag="f1")
                nc.tensor.matmul(f1[:], w1sT[:], xn[:, sl], start=True, stop=True)
                nc.scalar.activation(
                    f_aug[0:32, sl], f1[:], mybir.ActivationFunctionType.Relu,
                    bias=b1f[:],
                )

            # ---- kerT[q, pi] per block: (f_blk^T w2 + b2) * maskT ----
            for blk in range(NBLK):
                ksl = slice(blk * 49, (blk + 1) * 49)
                kps = pgen.tile([C, 512], f32, tag="kps")
                nc.tensor.matmul(
                    kps[:, 0:49], f_aug[:, blk * 128:(blk + 1) * 128], w2Ta[:],
                    start=True, stop=True,
                )
                nc.vector.tensor_mul(kerT[:, ksl], kps[:, 0:49], maskT[:, ksl])

            # ---- kerTswap: kerT with partition halves swapped (whole
            # tile; only the odd-shift columns are consumed) ----
            for h in range(2):
                nc.sync.dma_start(
                    kerTs[64 * h:64 * h + 64, :],
                    kerT[64 * (1 - h):64 * (1 - h) + 64, :],
                )

            # ---- involution: per block, batched diag builds + 49x2
            # accumulating [64x64] diag-matmuls ----
            for blk in range(NBLK):
                ops = pout.tile([C, 512], f32, tag="ops")
                opsw = pout.tile([C, 512], f32, tag="opsw")
                d_all = dpool.tile([C, 49 * 64], f16, tag="dall")
                nc.vector.tensor_mul(
                    d_all[:, 0:NEVEN * 64].rearrange("k (o j) -> k o j", j=64),
                    eye[:].unsqueeze(1).broadcast_to([C, NEVEN, 64]),
                    kerT[:, blk * 49: blk * 49 + NEVEN].unsqueeze(2)
                        .broadcast_to([C, NEVEN, 64]),
                )
                nc.vector.tensor_mul(
                    d_all[:, NEVEN * 64:(NEVEN + NDVODD) * 64]
                        .rearrange("k (o j) -> k o j", j=64),
                    eye[:].unsqueeze(1).broadcast_to([C, NDVODD, 64]),
                    kerTs[:, blk * 49 + NEVEN: blk * 49 + NEVEN + NDVODD]
                        .unsqueeze(2).broadcast_to([C, NDVODD, 64]),
                )
                for pi in range(NEVEN + NDVODD, 49):
                    nc.scalar.activation(
                        d_all[:, pi * 64:(pi + 1) * 64], eye[:],
                        mybir.ActivationFunctionType.Copy,
                        scale=kerTs[:, blk * 49 + pi: blk * 49 + pi + 1],
                    )
                for pi in range(49):
                    o = PERM[pi]
                    di, dj = divmod(o, 7)
                    sh = di - 3
                    odd = pi >= NEVEN
                    tgt = opsw if odd else ops
                    for rho in range(2):
                        rpp = (rho + sh) & 1
                        u = (rho + sh - rpp) >> 1     # in {-2..2}
                        xoff = (blk + u + 2) * 128
                        nc.tensor.matmul(
                            tgt[64 * rpp:64 * rpp + 64, 0:128],
                            d_all[64 * rpp:64 * rpp + 64,
                                  pi * 64:(pi + 1) * 64],
                            xtc[dj][64 * rpp:64 * rpp + 64,
                                    xoff: xoff + 128],
                            start=(pi == (NEVEN if odd else 0)),
                            stop=(pi == (48 if odd else NEVEN - 1)),
                            tile_position=(64 * rpp, 64 * rpp),
                        )
                nc.vector.tensor_copy(
                    outT[:, blk * 128:(blk + 1) * 128], ops[:, 0:128]
                )
                nc.vector.tensor_copy(
                    outT2[:, blk * 128:(blk + 1) * 128], opsw[:, 0:128]
                )

            nc.sync.dma_start(out_d, outT[:])
            nc.sync.dma_start(out2_d, outT2[:])

    nc.compile()
    return nc


def _get_nc():
    if "nc" not in _STATE:
        _STATE["nc"] = _build()
    return _STATE["nc"]


def _host_prep(x, w1, b1, bn_gamma, bn_beta, bn_mean, bn_var, w2, b2):
    x = np.asarray(x, dtype=np.float32)
    scale = np.asarray(bn_gamma) / np.sqrt(np.asarray(bn_var) + EPS)
    w1s = (np.asarray(w1) * scale[:, None]).astype(np.float32)
    b1f = (np.asarray(b1) * scale + np.asarray(bn_beta)
           - np.asarray(bn_mean) * scale).astype(np.float32)
    w1sT = np.ascontiguousarray(w1s.T.astype(np.float16))        # [128, 32]
    b1fc = np.ascontiguousarray(b1f[:, None].astype(np.float32))
    w2Ta_full = np.zeros((33, 49), dtype=np.float16)
    w2Ta_full[:32] = np.asarray(w2, np.float32).T.astype(np.float16)
    w2Ta_full[32] = np.asarray(b2, np.float32).astype(np.float16)
    w2Ta = np.ascontiguousarray(w2Ta_full[:, PERM])              # permuted

    # maskT[k, blk*49 + pi]: kernel zeroed where w + dj leaves the row
    wcol = np.arange(P, dtype=np.int64) % W
    mask = np.zeros((49, P), dtype=np.float16)
    for ipp in range(KK):
        for jpp in range(KK):
            dj = jpp - 3
            mask[ipp * KK + jpp] = ((wcol + dj >= 0) & (wcol + dj < W))
    mask = mask[PERM]                                            # permuted
    maskT = np.ascontiguousarray(
        mask.reshape(49, NBLK, 128).transpose(2, 1, 0).reshape(C, NBLK * 49)
    )

    eye = np.zeros((C, 64), dtype=np.float16)
    eye[np.arange(C), np.arange(C) % 64] = 1.0

    in_maps = []
    for core in range(8):
        b, half = divmod(core, 2)
        h0 = HH * half
        xn = np.ascontiguousarray(
            x[b, :, h0:h0 + HH, :].reshape(C, P).astype(np.float16)
        )
        # 7 shifted transposed copies: xt7[k, cpy*XBLK*128 + ib*128 + c]
        # holds x[b, c, strip_pixel (ib-2)*128 + k + (dj-3)]
        PAD = 6 * W
        xpadT = np.zeros((PAD + H * W + PAD, C), dtype=np.float16)
        xpadT[PAD:PAD + H * W] = x[b].reshape(C, H * W).T.astype(np.float16)
        xt7 = np.zeros((C, 7 * XBLK * 128), dtype=np.float16)
        base0 = PAD + h0 * W
        for dj in range(KK):
            st = base0 - 2 * 128 + (dj - 3)
            seg = xpadT[st: st + XBLK * 128]          # [XBLK*128, C]
            seg = np.ascontiguousarray(
                seg.reshape(XBLK, 128, C).transpose(1, 0, 2)
            )
            xt7[:, dj * XBLK * 128:(dj + 1) * XBLK * 128] = (
                seg.reshape(128, XBLK * 128)
            )
        in_maps.append({
            "xt7": xt7, "xn": xn, "w1sT": w1sT, "b1f": b1fc,
            "w2Ta": w2Ta, "maskT": maskT, "eyec": eye,
        })
    return in_maps


def run(inputs: dict, trace: bool = False):
    from concourse.bass_utils import run_bass_kernel_spmd

    nc = _get_nc()
    in_maps = _host_prep(**inputs)
    res = run_bass_kernel_spmd(
        nc, in_maps, core_ids=list(range(8)), trace=trace,
    )
    out = np.zeros((B, C, H, W), dtype=np.float32)
    for core in range(8):
        b, half = divmod(core, 2)
        h0 = HH * half
        arr = (res.results[core]["out"].astype(np.float32)
               + np.roll(res.results[core]["out2"].astype(np.float32),
                         64, axis=0))                  # [q, blk*128+c]
        arr = arr.reshape(128, NBLK, 128).transpose(1, 0, 2).reshape(P, C)
        out[b, :, h0:h0 + HH, :] = arr.T.reshape(C, HH, W)
    return out, res


def kernel(**inputs) -> np.ndarray:
    out, _ = run(inputs, trace=False)
    return out
